# revision 2
# baseline (speedup 1.0000x reference)
"""Pixelwise contrastive loss on 8 Trainium2 cores.

Phase A (per core k): two dma_gather calls (pixel index split at 32768 to fit
int16 indices; each pays the ~1us SWDGE launch once, vs 11x for per-tile
indirect DMA) pull the sampled pixel rows of the pixel-major [HW, C] map
slice into SBUF as [row-part, C-free] tiles. Square + per-tile 3D
tensor_reduce on DVE give the norms; ACT sqrt (table prefetched by a dummy
op); normalize muls split ACT/DVE; emit bf16 [128, NPAD]. Host glue
reassembles the global [C, 10240] normalized matrix (the "all-gather").

Phase B (per core k): 256 pos rows x 10240 cols of cosine similarity via PE
matmuls in 512-col PSUM pieces, grouped in [1024,1024,2048x4] column chunks
(small lead-in chunks so ACT starts early); one Exp per (chunk, row-group)
on ACT writing bf16; all row sums on DVE (2x bf16 mode). Host computes the
NLL tail in f64 from the per-row pos/total exp sums.
"""

import sys

if "/opt/trn_rl_repo" not in sys.path:
    sys.path.insert(0, "/opt/trn_rl_repo")

import numpy as np
import ml_dtypes

from concourse import bass, mybir, bass_utils
from concourse import bacc
import concourse.tile as tile

B, C, H, W = 8, 128, 256, 256
HW = H * W
SPLIT = HW // 2  # int16 index range per dma_gather
N_POS, N_NEG = 2048, 8192
NTOT = N_POS + N_NEG
NCORES = 8
BF16 = ml_dtypes.bfloat16
E1 = float(np.exp(np.float32(1.0)))

# Phase-B column chunks; first two sum to N_POS (the pos block boundary).
CHUNKS = (1024, 1024, 2048, 2048, 2048, 2048)
NCH = len(CHUNKS)
CH_OFF = tuple(int(x) for x in np.cumsum((0,) + CHUNKS))

_PROG_A = {}
_PROG_B = None


def _build_phase_a(NT_LO, NT_HI):
    NLO, NHI = NT_LO * 128, NT_HI * 128
    NT = NT_LO + NT_HI
    NPAD = NT * 128
    nc = bacc.Bacc("TRN2", target_bir_lowering=False)
    mapkT = nc.dram_tensor("mapkT", [HW, C], mybir.dt.float32, kind="ExternalInput")
    tblT = nc.dram_tensor("tbl", [128, NPAD // 16], mybir.dt.int16, kind="ExternalInput")
    xnT = nc.dram_tensor("xn", [128, NPAD], mybir.dt.bfloat16, kind="ExternalOutput")
    with tile.TileContext(nc) as tc:
        with tc.tile_pool(name="main", bufs=1) as pool:
            # sqrt-table prefetch: overlaps the ACT table load with the gathers
            dmy = pool.tile([128, 1], mybir.dt.float32)
            dmy2 = pool.tile([128, 1], mybir.dt.float32)
            nc.vector.memset(dmy[:], 1.0)
            nc.scalar.sqrt(dmy2[:], dmy[:])

            tbl_s = pool.tile([128, NPAD // 16], mybir.dt.int16)
            nc.sync.dma_start(out=tbl_s[:], in_=tblT[:])
            g = pool.tile([128, NPAD], mybir.dt.float32)
            sq = pool.tile([128, NPAD], mybir.dt.float32)
            n2 = pool.tile([128, NT], mybir.dt.float32)
            sections = ((0, NLO, 0, SPLIT), (NLO, NPAD, SPLIT, HW))
            for lo, hi, rlo, rhi in sections:
                nc.gpsimd.dma_gather(
                    g[:, lo:hi].rearrange("p (t e) -> p t e", e=128),
                    mapkT[rlo:rhi, :],
                    tbl_s[:, lo // 16:hi // 16],
                    hi - lo, hi - lo, C,
                )
            for lo, hi, _, _ in sections:
                nc.vector.tensor_tensor(
                    out=sq[:, lo:hi], in0=g[:, lo:hi], in1=g[:, lo:hi],
                    op=mybir.AluOpType.mult,
                )
                nc.vector.tensor_reduce(
                    out=n2[:, lo // 128:hi // 128],
                    in_=sq[:, lo:hi].rearrange("p (t e) -> p t e", e=128),
                    axis=mybir.AxisListType.X, op=mybir.AluOpType.add,
                )
            nrm = pool.tile([128, NT], mybir.dt.float32)
            nc.scalar.sqrt(nrm[:], n2[:])
            r1 = pool.tile([128, NT], mybir.dt.float32)
            nc.vector.reciprocal(out=r1[:], in_=nrm[:])
            # x / max(norm, 1e-6) == x * min(1/norm, 1e6)
            r3 = pool.tile([128, NT], mybir.dt.float32)
            nc.vector.tensor_scalar_min(out=r3[:], in0=r1[:], scalar1=1.0e6)
            xn = pool.tile([128, NPAD], mybir.dt.bfloat16)
            CHD = 4  # tiles per output DMA chunk
            for t in range(NT):
                xs = xn[:, t * 128:(t + 1) * 128]
                gs = g[:, t * 128:(t + 1) * 128]
                if t % 2 == 0:
                    nc.vector.tensor_scalar_mul(out=xs, in0=gs, scalar1=r3[:, t:t + 1])
                else:
                    nc.scalar.activation(
                        out=xs, in_=gs,
                        func=mybir.ActivationFunctionType.Copy,
                        scale=r3[:, t:t + 1],
                    )
                if t % CHD == CHD - 1 or t == NT - 1:
                    lo = (t // CHD) * CHD * 128
                    hi = (t + 1) * 128
                    nc.sync.dma_start(out=xnT[:, lo:hi], in_=xn[:, lo:hi])
    nc.finalize()
    return nc


def _build_phase_b():
    nc = bacc.Bacc("TRN2", target_bir_lowering=False)
    posT = nc.dram_tensor("posT", [128, 256], mybir.dt.bfloat16, kind="ExternalInput")
    colsT = nc.dram_tensor("cols", [128, NTOT], mybir.dt.bfloat16, kind="ExternalInput")
    sumsT = nc.dram_tensor("sums", [128, 2 * NCH], mybir.dt.float32, kind="ExternalOutput")
    with tile.TileContext(nc) as tc:
        with tc.tile_pool(name="main", bufs=1) as pool, \
             tc.tile_pool(name="ps", bufs=2, space="PSUM") as pool_ps, \
             tc.tile_pool(name="es", bufs=3) as pool_es:
            # exp-table prefetch: overlaps the ACT table load with input DMAs
            dmy = pool.tile([128, 1], mybir.dt.float32)
            dmy2 = pool.tile([128, 1], mybir.dt.float32)
            nc.vector.memset(dmy[:], 0.0)
            nc.scalar.activation(
                out=dmy2[:], in_=dmy[:], func=mybir.ActivationFunctionType.Exp
            )

            posT_s = pool.tile([128, 256], mybir.dt.bfloat16)
            nc.sync.dma_start(out=posT_s[:], in_=posT[:])
            cols_s = pool.tile([128, NTOT], mybir.dt.bfloat16)
            for b in range(NCH):
                sl = slice(CH_OFF[b], CH_OFF[b + 1])
                nc.sync.dma_start(out=cols_s[:, sl], in_=colsT[:, sl])
            sums_s = pool.tile([128, 2 * NCH], mybir.dt.float32)
            for b in range(NCH):
                csz = CHUNKS[b]
                for gg in range(2):
                    ps = pool_ps.tile([128, 2048], mybir.dt.float32)
                    for q in range(csz // 512):
                        nc.tensor.matmul(
                            out=ps[:, q * 512:(q + 1) * 512],
                            lhsT=posT_s[:, gg * 128:(gg + 1) * 128],
                            rhs=cols_s[:, CH_OFF[b] + q * 512:CH_OFF[b] + (q + 1) * 512],
                            start=True,
                            stop=True,
                        )
                    es = pool_es.tile([128, 2048], mybir.dt.bfloat16)
                    nc.scalar.activation(
                        out=es[:, :csz], in_=ps[:, :csz],
                        func=mybir.ActivationFunctionType.Exp,
                    )
                    nc.vector.tensor_reduce(
                        out=sums_s[:, gg * NCH + b:gg * NCH + b + 1],
                        in_=es[:, :csz],
                        axis=mybir.AxisListType.X, op=mybir.AluOpType.add,
                    )
            nc.sync.dma_start(out=sumsT[:], in_=sums_s[:])
    nc.finalize()
    return nc


def _get_out(core_results, key):
    if key in core_results:
        return np.asarray(core_results[key])
    return np.asarray(next(iter(core_results.values())))


def _wrap16(vals, npad):
    """[j%16, j//16] wrapped int16 index layout, replicated to 128 partitions;
    padded with index 0 (a valid row, so no SBUF garbage)."""
    e = np.zeros(npad, np.int64)
    e[:len(vals)] = vals
    m = e.reshape(npad // 16, 16).T.astype(np.int16)  # [16, npad//16]
    return np.tile(m, (8, 1))


def _run_all(inputs, trace=False):
    global _PROG_B
    psm = np.asarray(inputs["predict_seg_map"], dtype=np.float32)
    pb = np.asarray(inputs["pos_b"]).astype(np.int64)
    ph = np.asarray(inputs["pos_h"]).astype(np.int64)
    pw = np.asarray(inputs["pos_w"]).astype(np.int64)
    nb = np.asarray(inputs["neg_b"]).astype(np.int64)
    nh = np.asarray(inputs["neg_h"]).astype(np.int64)
    nw = np.asarray(inputs["neg_w"]).astype(np.int64)

    allb = np.concatenate([pb, nb])
    allpix = np.concatenate([ph * W + pw, nh * W + nw])
    gids = np.arange(NTOT, dtype=np.int64)

    ids_lo, pix_lo, ids_hi, pix_hi = [], [], [], []
    for k in range(NCORES):
        m = allb == k
        idk, pxk = gids[m], allpix[m]
        o = np.argsort(pxk, kind="stable")
        idk, pxk = idk[o], pxk[o]
        nlo = int(np.searchsorted(pxk, SPLIT))
        ids_lo.append(idk[:nlo])
        pix_lo.append(pxk[:nlo])
        ids_hi.append(idk[nlo:])
        pix_hi.append(pxk[nlo:] - SPLIT)
    NT_LO = max(1, max((len(x) + 127) // 128 for x in pix_lo))
    NT_HI = max(1, max((len(x) + 127) // 128 for x in pix_hi))
    NLO, NHI = NT_LO * 128, NT_HI * 128
    NT = NT_LO + NT_HI
    NPAD = NT * 128

    psmT = np.ascontiguousarray(psm.reshape(B, C, HW).transpose(0, 2, 1))
    tbls = [
        np.concatenate(
            [_wrap16(pix_lo[k], NLO), _wrap16(pix_hi[k], NHI)], axis=1
        )
        for k in range(NCORES)
    ]

    key = (NT_LO, NT_HI)
    if key not in _PROG_A:
        _PROG_A[key] = _build_phase_a(NT_LO, NT_HI)
    nc_a = _PROG_A[key]
    in_maps_a = [{"mapkT": psmT[k], "tbl": tbls[k]} for k in range(NCORES)]
    ra = bass_utils.run_bass_kernel_spmd(
        nc_a, in_maps_a, list(range(NCORES)), trace=trace
    )

    allN_T = np.zeros((NTOT, C), dtype=BF16)
    for k in range(NCORES):
        xnk = _get_out(ra.results[k], "xn")  # [128, NPAD]
        v = xnk.reshape(128, NT, 128).transpose(1, 0, 2).reshape(NPAD, 128)
        allN_T[ids_lo[k]] = v[:len(ids_lo[k])]
        allN_T[ids_hi[k]] = v[NLO:NLO + len(ids_hi[k])]
    cols = np.ascontiguousarray(allN_T.T)  # [C, NTOT]

    if _PROG_B is None:
        _PROG_B = _build_phase_b()
    in_maps_b = [
        {
            "posT": np.ascontiguousarray(cols[:, k * 256:(k + 1) * 256]),
            "cols": cols,
        }
        for k in range(NCORES)
    ]
    rb = bass_utils.run_bass_kernel_spmd(
        _PROG_B, in_maps_b, list(range(NCORES)), trace=trace
    )

    tot = 0.0
    for k in range(NCORES):
        sums = _get_out(rb.results[k], "sums").astype(np.float64)  # [128, 2*NCH]
        for gg in range(2):
            row = sums[:, gg * NCH:(gg + 1) * NCH]
            possum = row[:, :2].sum(axis=1)  # chunks 0,1 = pos columns
            total = row.sum(axis=1)
            tot += float(np.log((possum - E1) / (total - E1)).sum())
    nll = -tot / N_POS

    ns = None
    if trace:
        ns = (ra.exec_time_ns or 0) + (rb.exec_time_ns or 0)
    return np.float32(nll), ns


def kernel(predict_seg_map, pos_b, pos_h, pos_w, neg_b, neg_h, neg_w):
    out, _ = _run_all(
        {
            "predict_seg_map": predict_seg_map,
            "pos_b": pos_b, "pos_h": pos_h, "pos_w": pos_w,
            "neg_b": neg_b, "neg_h": neg_h, "neg_w": neg_w,
        },
        trace=False,
    )
    return np.asarray(out, dtype=np.float32)
